# revision 10
# baseline (speedup 1.0000x reference)
"""Multi-head self-attention TRN2 kernel (8 NeuronCores, head-parallel).

Problem: x[L=4096, N=1, E=1024], w_qkv[3E, E], w_out[E, E], H=16 heads, DH=64.
Sharding: 2 heads per core (128 q/k/v dims). Each core computes its heads'
attention and a partial out-projection; host sums the 8 partials.

Per-core algorithm (all matmuls bf16, fp32 PSUM accumulation):
  qT[d,l] = wqT.T @ xT   (scale folded into wq on host)
  kT[d,l] = wkT.T @ xT
  V[l,d]  = xT.T @ wvT   (stored as V_aug = [V_A|1|V_B|1] for the PV matmul)
  For each query chunk (512 cols) and each key tile jt (128 rows):
    ST[j, i]  = kT[:,jt].T @ qT[:,chunk]   (both heads row-tiled on the PE)
    PT        = exp(ST)                    (no max subtraction: |S| <= ~5)
    O_h[65,i] += [V_h|1].T @ PT_h          (row 64 = softmax denominator)
  OTn[d,i] = O_h[0:64] * broadcast(1/denom) ; out = OTn.T @ woT per l-tile.
"""

import sys
import os
import numpy as np

try:
    import concourse.bass as bass  # noqa: F401
except ImportError:
    sys.path.insert(0, "/opt/trn_rl_repo")

import ml_dtypes
import concourse.bass as bass
import concourse.mybir as mybir
import concourse.tile as tile
from concourse import bacc
from concourse.bass_utils import run_bass_kernel_spmd
from concourse.masks import make_identity

BF16 = mybir.dt.bfloat16
F32 = mybir.dt.float32
AF = mybir.ActivationFunctionType

L, N, E, H = 4096, 1, 1024, 16
DH = E // H            # 64
P = 128                # partitions / dims per core (2 heads)
SCALE = DH ** -0.5
NCORES = 8
ET = E // P            # 8 contraction tiles for the projections


def build(nc, L=L):
    LT = L // P            # key tiles
    CH = L // 512          # query chunks of 512
    CW = 512               # chunk width

    xT_d = nc.declare_dram_parameter("xT", [E, L], BF16, isOutput=False)
    wqT_d = nc.declare_dram_parameter("wqT", [E, P], BF16, isOutput=False)
    wkT_d = nc.declare_dram_parameter("wkT", [E, P], BF16, isOutput=False)
    wvT_d = nc.declare_dram_parameter("wvT", [E, P], BF16, isOutput=False)
    woT_d = nc.declare_dram_parameter("woT", [P, E], BF16, isOutput=False)
    out_d = nc.declare_dram_parameter("out", [L, E], BF16, isOutput=True)

    xT_t = xT_d.ap().rearrange("(t p) l -> t p l", p=P)
    wq_t = wqT_d.ap().rearrange("(t p) d -> t p d", p=P)
    wk_t = wkT_d.ap().rearrange("(t p) d -> t p d", p=P)
    wv_t = wvT_d.ap().rearrange("(t p) d -> t p d", p=P)
    out_t = out_d.ap().rearrange("(t p) f -> t p f", p=P)

    with tile.TileContext(nc) as tc:
        with (
            tc.tile_pool(name="persist", bufs=1) as sbp,
            tc.tile_pool(name="pt", bufs=3) as sb_pt,
            tc.tile_pool(name="ob", bufs=3) as sb_ob,
            tc.tile_pool(name="misc", bufs=2) as sb_misc,
            tc.tile_pool(name="psbig", bufs=2, space="PSUM") as ps_big,
            tc.tile_pool(name="psone", bufs=2, space="PSUM") as ps_one,
            tc.tile_pool(name="pso", bufs=2, space="PSUM") as ps_o,
        ):
            # ---- persistent SBUF tiles + input DMAs ----
            wq_sb, wk_sb, wv_sb = [], [], []
            for e in range(ET):
                for lst, src, nm in ((wq_sb, wq_t, "wq"), (wk_sb, wk_t, "wk"),
                                     (wv_sb, wv_t, "wv")):
                    t = sbp.tile([P, P], BF16, tag=f"{nm}{e}")
                    nc.sync.dma_start(out=t, in_=src[e])
                    lst.append(t)
            wo_sb = sbp.tile([P, E], BF16, tag="wo")
            nc.sync.dma_start(out=wo_sb, in_=woT_d.ap())
            ident = sbp.tile([P, P], BF16, tag="ident")
            make_identity(nc, ident)

            # xT loaded in column blocks so the projections can start after
            # the first block instead of after the full 8.4 MB transfer
            xT_sb = []
            for e in range(ET):
                t = sbp.tile([P, L], BF16, tag=f"xt{e}")
                xT_sb.append(t)
            for lc in range(CH):
                for e in range(ET):
                    nc.sync.dma_start(
                        out=xT_sb[e][:, lc * CW:(lc + 1) * CW],
                        in_=xT_t[e][:, lc * CW:(lc + 1) * CW])

            V_aug = []
            for lt in range(LT):
                t = sbp.tile([P, 2 * DH + 2], BF16, tag=f"va{lt}")
                nc.vector.memset(t[:, DH:DH + 1], 1.0)
                nc.vector.memset(t[:, 2 * DH + 1:2 * DH + 2], 1.0)
                V_aug.append(t)

            qT = sbp.tile([P, L], BF16, tag="qT")
            kT = sbp.tile([P, L], BF16, tag="kT")
            vT = sbp.tile([P, L], BF16, tag="vT")
            OTn = sbp.tile([P, L], BF16, tag="otn")

            # ---- phase 1: projections ----
            def proj_chunk(dst, w, lc):
                ps = ps_one.tile([P, CW], F32, tag="p1")
                for e in range(ET):
                    nc.tensor.matmul(
                        ps, lhsT=w[e], rhs=xT_sb[e][:, lc * CW:(lc + 1) * CW],
                        start=(e == 0), stop=(e == ET - 1))
                nc.vector.tensor_copy(out=dst[:, lc * CW:(lc + 1) * CW], in_=ps)

            for lc in range(CH):
                proj_chunk(kT, wk_sb, lc)
                proj_chunk(vT, wv_sb, lc)
                # V_aug[lt] = vT[:, lt*128:+128].T via PE transpose
                for lt in range(lc * (CW // P), (lc + 1) * (CW // P)):
                    tp = ps_one.tile([P, P], BF16, tag="p1")
                    nc.tensor.transpose(tp, vT[:, lt * P:(lt + 1) * P], ident)
                    nc.vector.tensor_copy(out=V_aug[lt][:, 0:DH], in_=tp[:, 0:DH])
                    nc.vector.tensor_copy(out=V_aug[lt][:, DH + 1:2 * DH + 1],
                                          in_=tp[:, DH:2 * DH])
            proj_chunk(qT, wq_sb, 0)

            # ---- phase 2: attention ----
            def emit_scores(c, jt):
                st = ps_big.tile([P, 2 * CW], F32, tag="st")
                nc.tensor.matmul(
                    st[:, 0:CW], lhsT=kT[0:DH, jt * P:(jt + 1) * P],
                    rhs=qT[0:DH, c * CW:(c + 1) * CW], start=True, stop=True)
                nc.tensor.matmul(
                    st[:, CW:2 * CW], lhsT=kT[DH:P, jt * P:(jt + 1) * P],
                    rhs=qT[DH:P, c * CW:(c + 1) * CW], start=True, stop=True)
                return st

            def outproj_unit(c, lt, fc):
                # out[l, f] for l-tile lt of chunk c, f columns [fc*512, +512)
                def emit():
                    glt = c * (CW // P) + lt
                    po = ps_one.tile([P, CW], F32, tag="p1")
                    nc.tensor.matmul(
                        po, lhsT=OTn[:, glt * P:(glt + 1) * P],
                        rhs=wo_sb[:, fc * CW:(fc + 1) * CW], start=True, stop=True)
                    ob = sb_ob.tile([P, CW], BF16, tag="ob")
                    nc.vector.tensor_copy(out=ob, in_=po)
                    nc.sync.dma_start(out=out_t[glt][:, fc * CW:(fc + 1) * CW],
                                      in_=ob)
                return emit

            deferred = []
            for c in range(CH):
                o_a = ps_o.tile([DH + 1, CW], F32, tag="o")
                o_b = ps_o.tile([DH + 1, CW], F32, tag="o")
                st_cur = emit_scores(c, 0)
                for jt in range(LT):
                    pt = sb_pt.tile([P, 2 * CW], BF16, tag="pt")
                    nc.scalar.activation(out=pt, in_=st_cur, func=AF.Exp)
                    if jt < LT - 1:
                        st_next = emit_scores(c, jt + 1)
                    nc.tensor.matmul(
                        o_a, lhsT=V_aug[jt][:, 0:DH + 1], rhs=pt[:, 0:CW],
                        start=(jt == 0), stop=(jt == LT - 1))
                    nc.tensor.matmul(
                        o_b, lhsT=V_aug[jt][:, DH + 1:2 * DH + 2],
                        rhs=pt[:, CW:2 * CW],
                        start=(jt == 0), stop=(jt == LT - 1))
                    if deferred and jt % 3 == 2:
                        deferred.pop(0)()
                    if jt == 10 and c + 1 < CH:
                        proj_chunk(qT, wq_sb, c + 1)
                    if jt < LT - 1:
                        st_cur = st_next

                # chunk epilogue: copy O to SBUF first (frees the PSUM banks
                # so the next chunk's PV can start; keeps the PE dense so the
                # HAM clock stays at 2.4 GHz), then normalize off-path.
                oa_sb = sb_misc.tile([DH + 1, CW], F32, tag="oasb")
                ob_sb = sb_misc.tile([DH + 1, CW], F32, tag="obsb")
                nc.vector.tensor_copy(out=oa_sb, in_=o_a)
                nc.vector.tensor_copy(out=ob_sb, in_=o_b)
                # denominator rows live at partition 64; custom-DVE ops can't
                # shift partitions, so DMA them to partition 0 first.
                dn = sb_misc.tile([1, 2 * CW], F32, tag="dn")
                nc.sync.dma_start(out=dn[:, 0:CW], in_=oa_sb[DH:DH + 1, :])
                nc.sync.dma_start(out=dn[:, CW:2 * CW], in_=ob_sb[DH:DH + 1, :])
                ra = sb_misc.tile([1, 2 * CW], F32, tag="ra")
                nc.vector.reciprocal_approx_fast(out=ra, in_=dn)
                # broadcast 1/denom (partition 0) across 64 partitions
                bc_sb = sb_misc.tile([DH, 2 * CW], F32, tag="bcsb")
                nc.gpsimd.partition_broadcast(bc_sb, ra)
                nc.vector.tensor_mul(
                    out=OTn[0:DH, c * CW:(c + 1) * CW],
                    in0=oa_sb[0:DH, :], in1=bc_sb[:, 0:CW])
                otb = sb_misc.tile([DH, CW], BF16, tag="otb")
                nc.vector.tensor_mul(out=otb, in0=ob_sb[0:DH, :],
                                     in1=bc_sb[:, CW:2 * CW])
                # partition shift 0:64 -> 64:128 via SBUF->SBUF DMA
                nc.sync.dma_start(out=OTn[DH:P, c * CW:(c + 1) * CW], in_=otb)

                for lt in range(CW // P):
                    for fc in range(E // CW):
                        deferred.append(outproj_unit(c, lt, fc))
            for f in deferred:
                f()
    nc.finalize()
    return nc


_built = {}


def _get_nc(l=L):
    if l not in _built:
        nc = bacc.Bacc()
        _built[l] = build(nc, l)
    return _built[l]


def _prep_inputs(x, w_qkv, w_out, l=L):
    x2 = np.asarray(x, dtype=np.float32).reshape(l, E)
    xT = np.ascontiguousarray(x2.T).astype(ml_dtypes.bfloat16)
    wq, wk, wv = w_qkv[0:E], w_qkv[E:2 * E], w_qkv[2 * E:3 * E]
    in_maps = []
    for c in range(NCORES):
        d0 = c * P
        in_maps.append({
            "xT": xT,
            "wqT": np.ascontiguousarray(
                (wq[d0:d0 + P] * SCALE).T).astype(ml_dtypes.bfloat16),
            "wkT": np.ascontiguousarray(wk[d0:d0 + P].T).astype(ml_dtypes.bfloat16),
            "wvT": np.ascontiguousarray(wv[d0:d0 + P].T).astype(ml_dtypes.bfloat16),
            "woT": np.ascontiguousarray(
                w_out[:, d0:d0 + P].T).astype(ml_dtypes.bfloat16),
        })
    return in_maps


def _run(x, w_qkv, w_out, l=L, **kw):
    nc = _get_nc(l)
    in_maps = _prep_inputs(x, w_qkv, w_out, l)
    res = run_bass_kernel_spmd(nc, in_maps, core_ids=list(range(NCORES)), **kw)
    acc = np.zeros((l, E), dtype=np.float32)
    for r in res.results:
        acc += r["out"].astype(np.float32)
    return acc.reshape(l, N, E), res


def kernel(x, w_qkv, w_out):
    out, _ = _run(x, w_qkv, w_out)
    return out
